# revision 18
# baseline (speedup 1.0000x reference)
"""NetVLAD Trainium2 Bass kernel.

Full inputs in, full output out. Data-parallel over batch N=64 across 8
NeuronCores (8 samples per core); conv weight and centroids replicated.

Per-sample algorithm (mathematically equal to the reference, never
materializing the channel-normalized x):
  X = x[n]  [D=128, P=4800]  (D on SBUF partitions, contiguous in HBM)
  For each 128-wide chunk of P (p on partitions after a PE transpose):
    ss[p]    = sum_d X[d,p]^2
    inv_s    = ss^-0.5
    logitsT  = X_c^T @ Wt                (PE, fused with the transpose)
    e        = exp(logitsT * inv_s)      (softmax max-subtraction skipped:
                                          |logits*inv_s| <= ~1.2)
    sb       = e * (inv_s / Z),  Z = sum_k e
    acc     += sb^T @ [s | X_c^T]        (PE, PSUM accumulate)
  agg      = acc[:, 1:129];  sum_sa = acc[:, 0]
  vlad     = agg - sum_sa * centroids, then intra + global L2 norm.

Engine/dtype structure (tuned against the instruction-cost model):
 - x is converted to bf16 on the host: halves the HBM traffic (the
   memory roofline) and makes pass A an all-bf16 matmul with a 192-col
   [I128 | Wt64] moving tile (1 cycle/row, no fp32r 256-col pad).
 - the softmax-side tensors (logits/slgt/et/sbt) are K-MAJOR
   [128, K, NCH] so the per-(p,chunk) broadcast factors (inv_s, 1/Z)
   enter tensor_tensor as a step-0 MIDDLE dim with a packed bf16 last
   dim -> DVE runs them in 2x mode. (The baseline's step-0 LAST-dim
   broadcast disabled 2x.)
 - reductions (sum over d for ss, sum over k for Z) are pure
   tensor_tensor halving trees in bf16 (2x) instead of tensor_reduce
   (which has no fast mode).
 - PSUM evacuation is split: X^T part on ACT, logits part (k-major
   rearrange) on Pool, to balance ACT/DVE/Pool occupancy.
"""

import sys

if "/opt/trn_rl_repo" not in sys.path:
    sys.path.insert(0, "/opt/trn_rl_repo")

import numpy as np
from contextlib import ExitStack

N, D, HW, K = 64, 128, 4800, 64
NCORES = 8
NS = N // NCORES  # samples per core

CHUNKS = [(i * 128, min(128, HW - i * 128)) for i in range((HW + 127) // 128)]
NCH = len(CHUNKS)  # 38: 37 full + one 64-wide
NCHP = 40  # padded chunk-dim stride (80B, 8B-aligned) for k-major tiles

_CACHE = {}


def _patch_act_tables():
    """Steer bacc's ACT table-set placement to the one set that covers
    every function we use (ln/exp/square/copy) so the kernel pays a single
    ACT_TABLE_LOAD instead of thrashing between per-anchor sets."""
    if _CACHE.get("act_patched"):
        return
    from concourse import bacc, mybir

    orig = bacc.get_activation_tables
    AF = mybir.ActivationFunctionType
    combo = "natural_log_exp_and_others"

    def patched(arch):
        t = {k: set(v) for k, v in orig(arch).items()}
        if combo in t:
            for name in t:
                if name != combo:
                    t[name] = t[name] - {AF.Ln, AF.Exp, AF.Copy, AF.Square}
        return t

    bacc.get_activation_tables = patched
    _CACHE["act_patched"] = True


def _build_nc():
    import concourse.tile as tile
    from concourse import bacc, mybir

    _patch_act_tables()

    nc = bacc.Bacc(
        "TRN2",
        target_bir_lowering=False,
        debug=False,
        enable_asserts=False,
        num_devices=NCORES,
    )
    x_ap = nc.dram_tensor(
        "x", [NS, D, HW], mybir.dt.bfloat16, kind="ExternalInput"
    ).ap()
    idw_ap = nc.dram_tensor(
        "idw", [D, 192], mybir.dt.bfloat16, kind="ExternalInput"
    ).ap()
    id8_ap = nc.dram_tensor(
        "id8", [NS, NS], mybir.dt.float32, kind="ExternalInput"
    ).ap()
    cent_ap = nc.dram_tensor(
        "cent", [K, D], mybir.dt.float32, kind="ExternalInput"
    ).ap()
    out_ap = nc.dram_tensor(
        "out", [NS, K, D], mybir.dt.float32, kind="ExternalOutput"
    ).ap()

    with tile.TileContext(nc) as tc:
        with ExitStack() as ctx:
            _body(ctx, tc, out_ap, x_ap, cent_ap, idw_ap, id8_ap)
    nc.compile()
    return nc


def _body(ctx, tc, out_ap, x_ap, cent_ap, idw_ap, id8_ap):
    import concourse.bass as bass
    from concourse import mybir

    nc = tc.nc
    f32 = mybir.dt.float32
    f32r = mybir.dt.float32r
    bf16 = mybir.dt.bfloat16
    AF = mybir.ActivationFunctionType
    ALU = mybir.AluOpType
    X_AX = mybir.AxisListType.X

    singles = ctx.enter_context(tc.tile_pool(name="singles", bufs=1))
    xpool = ctx.enter_context(tc.tile_pool(name="xpool", bufs=3))
    xtrpool = ctx.enter_context(tc.tile_pool(name="xtrpool", bufs=4))
    lgtpool = ctx.enter_context(tc.tile_pool(name="lgtpool", bufs=2))
    ebpool = ctx.enter_context(tc.tile_pool(name="ebpool", bufs=2))
    sbtpool = ctx.enter_context(tc.tile_pool(name="sbtpool", bufs=2))
    scrpool = ctx.enter_context(tc.tile_pool(name="scrpool", bufs=2))
    smalls = ctx.enter_context(tc.tile_pool(name="smalls", bufs=6))
    tails = ctx.enter_context(tc.tile_pool(name="tails", bufs=1))
    pp_xt = ctx.enter_context(tc.tile_pool(name="pp_xt", bufs=2, space="PSUM"))
    pp_acc = ctx.enter_context(tc.tile_pool(name="pp_acc", bufs=1, space="PSUM"))
    pp_tiny = ctx.enter_context(tc.tile_pool(name="pp_tiny", bufs=1, space="PSUM"))

    def bcast(ap, n):
        # append a step-0 free dim: [..., n] broadcast view
        return bass.AP(tensor=ap.tensor, offset=ap.offset, ap=list(ap.ap) + [[0, n]])

    def mid_bcast(ap, n):
        # [p, f] -> [p, n, f] with step-0 middle dim (keeps last dim packed
        # so DVE 2x mode stays enabled)
        return bass.AP(
            tensor=ap.tensor,
            offset=ap.offset,
            ap=[ap.ap[0], [0, n]] + list(ap.ap[1:]),
        )

    # constants
    # fused rhs for pass A: [identity | Wt] bf16 — one matmul yields
    # [X_c^T | logits]. bf16 moving tensor streams at 1 cycle/row with no
    # 256-col fp32r padding requirement; the f32r lhsT (X chunk) keeps X
    # exact through the transpose path.
    identwt = singles.tile([128, 192], bf16)
    nc.sync.dma_start(out=identwt[:], in_=idw_ap[:])
    id8 = singles.tile([NS, NS], f32)
    nc.sync.dma_start(out=id8[:], in_=id8_ap[:])
    cent_s = singles.tile([K, D], f32)
    nc.sync.dma_start(out=cent_s[:], in_=cent_ap[:])
    ones_col = singles.tile([K, 1], f32)
    nc.vector.memset(ones_col[:], 1.0)
    ones_row = singles.tile([1, K], f32)
    nc.vector.memset(ones_row[:], 1.0)

    # 6 chunks per PSUM tile, rows padded to 256 f32: a matmul out that
    # crosses a 2KB PSUM bank boundary mid-row ACCUMULATES onto stale bank
    # contents instead of resetting on start=True (verified on HW), so every
    # out row must sit inside a 512-f32 bank. 256-aligned rows never cross.
    GRP = 6
    groups = []
    c0 = 0
    while c0 < NCH:
        groups.append(list(range(c0, min(c0 + GRP, NCH))))
        c0 += GRP
    NG = len(groups)  # 7

    # pass-C interleave: chunk range emitted after group g of the round
    pc_slices = []
    base = 0
    for g in range(NG):
        take = (NCH - base + (NG - g) - 1) // (NG - g)
        pc_slices.append((base, base + take))
        base += take

    state = {}  # per-sample live tiles

    def emit_dma(n):
        # 64-col pad past HW so the last (64-wide) chunk's pass-A matmul
        # still writes all 128 PSUM partitions: without this, stale PSUM
        # (possibly NaN from a previous process) flows into the evacuated
        # lanes. The pad x-vectors are all-1 (finite everywhere downstream);
        # pass C only ever reads the valid 64 rows of that chunk.
        xs = xpool.tile([D, HW + 64], bf16, tag="xs")
        nc.gpsimd.memset(xs[:, HW : HW + 64], 1.0)
        if n == 0:
            # per-group pieces: group-0 matmuls start after ~1/7 of the load
            for grp in groups:
                p0 = CHUNKS[grp[0]][0]
                p1 = CHUNKS[grp[-1]][0] + CHUNKS[grp[-1]][1]
                nc.sync.dma_start(out=xs[:, p0:p1], in_=x_ap[n, :, p0:p1])
        else:
            nc.sync.dma_start(out=xs[:, 0 : HW // 2], in_=x_ap[n, :, 0 : HW // 2])
            nc.sync.dma_start(out=xs[:, HW // 2 : HW], in_=x_ap[n, :, HW // 2 :])
        state.setdefault(n, {})["xs"] = xs

    def alloc_sample(n):
        st = state.setdefault(n, {})
        # per chunk (bf16): cols 0:128 = X_c^T, col 128 = ||x_p|| (s),
        # cols 129:136 pad the row stride to 272B (16B-aligned): the DVE
        # 2-elem/cycle mode needs 8B-aligned row starts, so a 130-elem
        # (260B) stride would drop the square to 1x. pass C's rhs is the
        # contiguous [X^T | s] 129-col slice.
        st["xtr"] = xtrpool.tile([128, NCH, 136], bf16, tag="xtr", name="xtr")
        # logits, K-major [p, k, chunk] with the chunk dim padded to
        # NCHP=40 (80B stride, 8B-aligned): the per-(p,chunk) broadcast
        # factors (inv_s, inv_s/Z) then enter tensor_tensor as a step-0
        # MIDDLE dim with a packed last dim, keeping DVE 2x alive. pass C
        # still needs a chunk-major sbt (contiguous lhsT), made by one
        # cheap transposed-view tensor_copy at the end.
        st["lgt"] = lgtpool.tile([128, K, NCHP], bf16, tag="lgt", name="lgt")

    def emit_passA_group(n, g):
        st = state[n]
        xs = st["xs"]
        grp = groups[g]
        xt_p = pp_xt.tile([128, GRP, 256], f32, tag="xt")
        for j, c in enumerate(grp):
            p0, _ = CHUNKS[c]
            # always full 128-wide (xs is padded): same stream cost, and
            # every PSUM partition the evac reads is freshly written
            nc.tensor.matmul(
                xt_p[:, j, 0:192],
                lhsT=xs[:, p0 : p0 + 128],
                rhs=identwt[:],
                start=True,
                stop=True,
            )
        return xt_p, grp[0], len(grp)

    def emit_evacs(n, g, xt_p, gc, gn):
        st = state[n]
        # X^T -> xtr cols 0:128 (ACT); logits -> lgt k-major. GPSIMD cannot
        # read PSUM, so the logits copies split across ACT and DVE to
        # balance those two queues.
        nc.scalar.copy(st["xtr"][:, gc : gc + gn, 0:128], xt_p[:, 0:gn, 0:128])
        lsrc = xt_p[:, 0:gn, 128:192].rearrange("p g k -> p k g")
        if g < 4:
            nc.scalar.copy(st["lgt"][:, :, gc : gc + gn], lsrc)
        else:
            nc.vector.tensor_copy(st["lgt"][:, :, gc : gc + gn], lsrc)

    # ---- stage B: ss = rowsum(X_c^T^2), inv_s, s-col (sample lag 1) ----
    def st_sq(n):
        st = state[n]
        x2t = scrpool.tile([128, NCH, 128], bf16, tag="x2t", bufs=1)
        st["x2t"] = x2t
        xv = st["xtr"][:, :, 0:128]
        nc.vector.tensor_tensor(out=x2t[:], in0=xv, in1=xv, op=ALU.mult)

    def st_tree123(n):
        st = state[n]
        x2t = st["x2t"]
        t1 = scrpool.tile([128, NCH, 64], bf16, tag="t1", bufs=2)
        t2 = scrpool.tile([128, NCH, 32], bf16, tag="t2", bufs=2)
        t3 = scrpool.tile([128, NCH, 16], bf16, tag="t3", bufs=2)
        st["t3"] = t3
        nc.vector.tensor_tensor(
            out=t1[:], in0=x2t[:, :, 0:64], in1=x2t[:, :, 64:128], op=ALU.add
        )
        nc.vector.tensor_tensor(
            out=t2[:], in0=t1[:, :, 0:32], in1=t1[:, :, 32:64], op=ALU.add
        )
        nc.vector.tensor_tensor(
            out=t3[:], in0=t2[:, :, 0:16], in1=t2[:, :, 16:32], op=ALU.add
        )

    def st_tree4567(n):
        st = state[n]
        t3 = st["t3"]
        t4 = scrpool.tile([128, NCH, 8], bf16, tag="t4", bufs=2)
        t5 = scrpool.tile([128, NCH, 4], bf16, tag="t5", bufs=2)
        t6 = scrpool.tile([128, NCH, 2], bf16, tag="t6", bufs=2)
        ss = smalls.tile([128, NCH], f32, tag="ss")
        st["ss"] = ss
        nc.vector.tensor_tensor(
            out=t4[:], in0=t3[:, :, 0:8], in1=t3[:, :, 8:16], op=ALU.add
        )
        nc.vector.tensor_tensor(
            out=t5[:], in0=t4[:, :, 0:4], in1=t4[:, :, 4:8], op=ALU.add
        )
        nc.vector.tensor_tensor(
            out=t6[:], in0=t5[:, :, 0:2], in1=t5[:, :, 2:4], op=ALU.add
        )
        nc.vector.tensor_tensor(
            out=ss[:], in0=t6[:, :, 0], in1=t6[:, :, 1], op=ALU.add
        )

    def st_ischain(n):
        st = state[n]
        lns = smalls.tile([128, NCH], f32, tag="lns")
        isb = smalls.tile([128, NCH], bf16, tag="isb")
        st["isb"] = isb
        nc.scalar.activation(lns[:], st["ss"][:], AF.Ln)
        nc.scalar.activation(isb[:], lns[:], AF.Exp, scale=-0.5)

    def st_scol(n):
        st = state[n]
        # s = ss * inv_s = ||x_p||, into col 128 of each xtr chunk
        nc.gpsimd.tensor_tensor(
            out=st["xtr"][:, :, 128], in0=st["ss"][:], in1=st["isb"][:], op=ALU.mult
        )

    # ---- stage C: scaled logits, K-major (Pool: its only sizable
    # SBUF-only job; DVE/ACT are saturated by evac+trees) ----
    def st_slg(n):
        st = state[n]
        st["slgt"] = scrpool.tile([128, K, NCHP], bf16, tag="slgt", name="slgt", bufs=2)
        nc.gpsimd.tensor_tensor(
            out=st["slgt"][:, :, 0:NCH],
            in0=st["lgt"][:, :, 0:NCH],
            in1=mid_bcast(st["isb"][:], K),
            op=ALU.mult,
        )

    # ---- stage D: exp (one whole-sample ACT instruction) ----
    def st_exp(n):
        st = state[n]
        st["et"] = ebpool.tile([128, K, NCHP], bf16, tag="et", name="et")
        nc.scalar.activation(
            st["et"][:, :, 0:NCH], st["slgt"][:, :, 0:NCH], AF.Exp
        )

    # ---- stage E: Z (k-halving tree), 1/Z, t = inv_s/Z, sb (lag 2) ----
    def st_zchain(n):
        st = state[n]
        et = st["et"]
        z1 = scrpool.tile([128, 32, NCHP], bf16, tag="z1", bufs=2)
        z2 = scrpool.tile([128, 16, NCHP], bf16, tag="z2", bufs=2)
        z3 = scrpool.tile([128, 8, NCHP], bf16, tag="z3", bufs=2)
        z4 = scrpool.tile([128, 4, NCHP], bf16, tag="z4", bufs=2)
        z5 = scrpool.tile([128, 2, NCHP], bf16, tag="z5", bufs=2)
        zz = smalls.tile([128, NCH], f32, tag="zz")
        st["zz"] = zz
        nc.vector.tensor_tensor(
            out=z1[:, :, 0:NCH], in0=et[:, 0:32, 0:NCH], in1=et[:, 32:64, 0:NCH], op=ALU.add
        )
        nc.vector.tensor_tensor(
            out=z2[:, :, 0:NCH], in0=z1[:, 0:16, 0:NCH], in1=z1[:, 16:32, 0:NCH], op=ALU.add
        )
        nc.vector.tensor_tensor(
            out=z3[:, :, 0:NCH], in0=z2[:, 0:8, 0:NCH], in1=z2[:, 8:16, 0:NCH], op=ALU.add
        )
        nc.vector.tensor_tensor(
            out=z4[:, :, 0:NCH], in0=z3[:, 0:4, 0:NCH], in1=z3[:, 4:8, 0:NCH], op=ALU.add
        )
        nc.vector.tensor_tensor(
            out=z5[:, :, 0:NCH], in0=z4[:, 0:2, 0:NCH], in1=z4[:, 2:4, 0:NCH], op=ALU.add
        )
        nc.vector.tensor_tensor(
            out=zz[:], in0=z5[:, 0, 0:NCH], in1=z5[:, 1, 0:NCH], op=ALU.add
        )

    def st_recip(n):
        st = state[n]
        rr = smalls.tile([128, NCH], f32, tag="rr")
        st["rr"] = rr
        nc.vector.reciprocal(rr[:], st["zz"][:])

    def st_tsc(n):
        st = state[n]
        tsc = smalls.tile([128, NCH], bf16, tag="tsc")
        st["tsc"] = tsc
        nc.vector.tensor_tensor(
            out=tsc[:], in0=st["isb"][:], in1=st["rr"][:], op=ALU.mult
        )

    def st_sbt(n):
        st = state[n]
        skm = sbtpool.tile([128, K, NCHP], bf16, tag="sbt_km", name="sbt_km")
        nc.vector.tensor_tensor(
            out=skm[:, :, 0:NCH],
            in0=st["et"][:, :, 0:NCH],
            in1=mid_bcast(st["tsc"][:], K),
            op=ALU.mult,
        )
        # chunk-major copy for pass C's contiguous lhsT (2x_2p tensor_copy)
        st["sbt"] = sbtpool.tile([128, NCH, K], bf16, tag="sbt", name="sbt")
        nc.vector.tensor_copy(
            st["sbt"][:], skm[:, :, 0:NCH].rearrange("p k c -> p c k")
        )

    cstate = {}  # open accumulation tiles for interleaved pass C

    def emit_passC_chunks(n, c0, c1):
        st = state[n]
        xtr, sbt = st["xtr"], st["sbt"]
        if n not in cstate:
            cstate[n] = pp_acc.tile([K, 129], f32, tag="acc", name="acc")
        acc_p = cstate[n]
        for c in range(c0, min(c1, NCH)):
            p0, w = CHUNKS[c]
            nc.tensor.matmul(
                acc_p[:, :],
                lhsT=sbt[:w, c, :],
                rhs=xtr[:w, c, 0:129],
                start=(c == 0),
                stop=(c == NCH - 1),
            )

    def finish_passC(n):
        acc_p = cstate.pop(n)
        state.pop(n)
        # evacuate [sum_sa | agg] in one copy
        nc.scalar.copy(comb_all[:, n, :], acc_p[:, 0 : D + 1])

    # batched across all samples; acc layout is [agg | sum_sa]
    comb_all = tails.tile([K, NS, D + 1], f32)
    agg_all = comb_all[:, :, 0:D]
    ssa_all = comb_all[:, :, D]

    def emit_tail(n0, n1):
        nn = n1 - n0
        agg_h = agg_all[:, n0:n1, :]
        ssa_h = ssa_all[:, n0:n1]
        vl = tails.tile([K, nn, D], f32, tag="t_vl", bufs=2)
        vsq = tails.tile([K, nn * D], f32, tag="t_vsq", bufs=2)
        q = tails.tile([K, nn], f32, tag="t_q", bufs=2)
        qm = tails.tile([K, nn], f32, tag="t_qm", bufs=2)
        isq = tails.tile([K, nn], f32, tag="t_isq", bufs=2)
        isq2 = tails.tile([K, nn], f32, tag="t_isq2", bufs=2)
        u = tails.tile([K, nn], f32, tag="t_u", bufs=2)
        gisr = tails.tile([1, nn], f32, tag="t_gisr", bufs=2)
        gb = tails.tile([K, nn], f32, tag="t_gb", bufs=2)
        sall = tails.tile([K, nn], f32, tag="t_s", bufs=2)
        vf = tails.tile([K, nn, D], f32, tag="t_vf", bufs=2)

        # vl = agg - ssa * cent
        nc.gpsimd.tensor_tensor(
            out=vl[:], in0=bcast(ssa_h, D), in1=mid_bcast(cent_s[:], nn), op=ALU.mult
        )
        nc.vector.tensor_tensor(out=vl[:], in0=agg_h, in1=vl[:], op=ALU.subtract)
        # q = rowsum(vl^2) per (k, n)
        vsqv = vsq[:].rearrange("k (n d) -> k n d", n=nn)
        nc.scalar.activation(vsqv, vl[:], AF.Square)
        nc.vector.tensor_reduce(out=q[:], in_=vsqv, axis=X_AX, op=ALU.add)
        nc.vector.tensor_scalar_max(qm[:], q[:], 1e-24)
        lq = tails.tile([K, nn], f32, tag="t_lq", bufs=2)
        nc.scalar.activation(lq[:], qm[:], AF.Ln)
        nc.scalar.activation(isq[:], lq[:], AF.Exp, scale=-0.5)
        # g = sum_k q_k * isq_k^2  (per sample)
        nc.vector.tensor_tensor(out=isq2[:], in0=isq[:], in1=isq[:], op=ALU.mult)
        nc.vector.tensor_tensor(out=u[:], in0=q[:], in1=isq2[:], op=ALU.mult)
        g_p = pp_tiny.tile([NS, 1], f32, tag="tiny")
        nc.tensor.matmul(
            g_p[:nn, :], lhsT=u[:], rhs=ones_col[:], start=True, stop=True
        )
        # gis = g^-0.5 -> transpose to a row -> broadcast over k partitions
        gm = tails.tile([nn, 1], f32, tag="t_gm", bufs=2)
        nc.vector.tensor_scalar_max(gm[:], g_p[:nn, :], 1e-24)
        gis = tails.tile([nn, 1], f32, tag="t_gis", bufs=2)
        lgm = tails.tile([nn, 1], f32, tag="t_lgm", bufs=2)
        nc.scalar.activation(lgm[:], gm[:], AF.Ln)
        nc.scalar.activation(gis[:], lgm[:], AF.Exp, scale=-0.5)
        gr_p = pp_tiny.tile([1, NS], f32, tag="tiny")
        nc.tensor.matmul(
            gr_p[:, :nn],
            lhsT=gis[:],
            rhs=id8[:nn, 0:nn],
            start=True,
            stop=True,
        )
        nc.vector.tensor_copy(gisr[:], gr_p[:, :nn])
        gb_p = pp_tiny.tile([K, NS], f32, tag="tiny")
        nc.tensor.matmul(
            gb_p[:, :nn], lhsT=ones_row[:], rhs=gisr[:], start=True, stop=True
        )
        nc.vector.tensor_copy(gb[:], gb_p[:, :nn])
        # s = isq * gb; vf = vl * s
        nc.vector.tensor_tensor(out=sall[:], in0=isq[:], in1=gb[:], op=ALU.mult)
        nc.gpsimd.tensor_tensor(out=vf[:], in0=vl[:], in1=bcast(sall[:], D), op=ALU.mult)
        nc.sync.dma_start(
            out=out_ap.rearrange("n k d -> k n d")[:, n0:n1, :], in_=vf[:]
        )

    # 5-stage software pipeline, one stage lag per round; every stage
    # consumes only tensors finished a round earlier:
    #   round r: DMA prefetch(r+2) | passA+evacs(r) | B: ss/inv_s/slg(r-1) |
    #   D: exp(r-2, ACT) | E: Z/recip/sb(r-2, DVE) | passC(r-3) on PE.
    emit_dma(0)
    emit_dma(1)
    pending_fin = None
    for r in range(NS + 3):
        na = r if r < NS else None
        if na is not None and r + 2 < NS:
            emit_dma(r + 2)
        if na is not None:
            alloc_sample(na)
        nb = r - 1 if 0 <= r - 1 < NS else None
        sd = r - 2 if 0 <= r - 2 < NS else None
        pe = r - 3 if 0 <= r - 3 < NS else None
        # finish the previous round's accumulation at the HEAD of this
        # round's queues so the acc bank frees before passC(pe) needs it
        if pending_fin is not None:
            finish_passC(pending_fin)
            if pending_fin % 2 == 1:
                emit_tail(pending_fin - 1, pending_fin + 1)
            pending_fin = None
        for g in range(NG):
            # passC first: it is always ready (sbt a round old), so the PE
            # drains it while a late xs DMA finishes instead of idling
            if pe is not None:
                emit_passC_chunks(pe, *pc_slices[g])
            if na is not None:
                xt_p, gc, gn = emit_passA_group(na, g)
                emit_evacs(na, g, xt_p, gc, gn)
            if g == 0 and nb is not None:
                st_sq(nb)
            elif g == 1 and nb is not None:
                st_tree123(nb)
            elif g == 2 and nb is not None:
                st_tree4567(nb)
            elif g == 3:
                if nb is not None:
                    st_ischain(nb)
                    st_scol(nb)
            elif g == 5:
                if sd is not None:
                    st_exp(sd)
                if nb is not None:
                    st_slg(nb)
        if sd is not None:
            # round tail: exp(sd) just finished on ACT; chain Z -> 1/Z -> t
            # -> sb here so sb is ready when passC(sd) starts next round
            st_zchain(sd)
            st_recip(sd)
            st_tsc(sd)
            st_sbt(sd)
        if pe is not None:
            pending_fin = pe
    if pending_fin is not None:
        n_last = pending_fin
        finish_passC(n_last)
        emit_tail(n_last - 1, n_last + 1)


def kernel(x, conv_w, centroids):
    from concourse.bass_utils import run_bass_kernel_spmd
    import ml_dtypes

    if "nc" not in _CACHE:
        _CACHE["nc"] = _build_nc()
    nc = _CACHE["nc"]

    x = np.ascontiguousarray(
        np.asarray(x, dtype=np.float32).reshape(N, D, HW).astype(ml_dtypes.bfloat16)
    )
    wt = np.ascontiguousarray(np.asarray(conv_w, dtype=np.float32).T)
    cent = np.ascontiguousarray(np.asarray(centroids, dtype=np.float32))
    idw = np.zeros((D, 192), dtype=np.float32)
    idw[:, 0:128] = np.eye(D, dtype=np.float32)
    idw[:, 128:192] = wt
    idw = idw.astype(ml_dtypes.bfloat16)
    id8 = np.eye(NS, dtype=np.float32)
    in_maps = [
        {"x": x[i * NS : (i + 1) * NS], "cent": cent, "idw": idw, "id8": id8}
        for i in range(NCORES)
    ]
    res = run_bass_kernel_spmd(nc, in_maps, core_ids=list(range(NCORES))).results
    out = np.concatenate([r["out"].reshape(NS, K * D) for r in res], axis=0)
    return out


if __name__ == "__main__":
    rng = np.random.default_rng(0)
    xs = rng.standard_normal((N, D, 60, 80), dtype=np.float32)
    cw = (rng.standard_normal((K, D)) * 0.1).astype(np.float32)
    ct = rng.random((K, D), dtype=np.float32)
    o = kernel(x=xs, conv_w=cw, centroids=ct)
    print("kernel out", o.shape, o.dtype, np.abs(o).max())


# revision 19
# speedup vs baseline: 1.1004x; 1.1004x over previous
"""NetVLAD Trainium2 Bass kernel.

Full inputs in, full output out. Data-parallel over batch N=64 across 8
NeuronCores (8 samples per core); conv weight and centroids replicated.

Per-sample algorithm (mathematically equal to the reference, never
materializing the channel-normalized x):
  X = x[n]  [D=128, P=4800]  (D on SBUF partitions, contiguous in HBM)
  For each 128-wide chunk of P (p on partitions after a PE transpose):
    ss[p]    = sum_d X[d,p]^2
    inv_s    = ss^-0.5
    logitsT  = X_c^T @ Wt                (PE, fused with the transpose)
    e        = exp(logitsT * inv_s)      (softmax max-subtraction skipped:
                                          |logits*inv_s| <= ~1.2)
    sb       = e * (inv_s / Z),  Z = sum_k e
    acc     += sb^T @ [s | X_c^T]        (PE, PSUM accumulate)
  agg      = acc[:, 1:129];  sum_sa = acc[:, 0]
  vlad     = agg - sum_sa * centroids, then intra + global L2 norm.

Engine/dtype structure (tuned against the instruction-cost model):
 - x is converted to bf16 on the host: halves the HBM traffic (the
   memory roofline) and makes pass A an all-bf16 matmul with a 192-col
   [I128 | Wt64] moving tile (1 cycle/row, no fp32r 256-col pad).
 - the softmax-side tensors (logits/slgt/et/sbt) are K-MAJOR
   [128, K, NCH] so the per-(p,chunk) broadcast factors (inv_s, 1/Z)
   enter tensor_tensor as a step-0 MIDDLE dim with a packed bf16 last
   dim -> DVE runs them in 2x mode. (The baseline's step-0 LAST-dim
   broadcast disabled 2x.)
 - reductions (sum over d for ss, sum over k for Z) are pure
   tensor_tensor halving trees in bf16 (2x) instead of tensor_reduce
   (which has no fast mode).
 - PSUM evacuation is split: X^T part on ACT, logits part (k-major
   rearrange) on Pool, to balance ACT/DVE/Pool occupancy.
"""

import sys

if "/opt/trn_rl_repo" not in sys.path:
    sys.path.insert(0, "/opt/trn_rl_repo")

import numpy as np
from contextlib import ExitStack

N, D, HW, K = 64, 128, 4800, 64
NCORES = 8
NS = N // NCORES  # samples per core

CHUNKS = [(i * 128, min(128, HW - i * 128)) for i in range((HW + 127) // 128)]
NCH = len(CHUNKS)  # 38: 37 full + one 64-wide
NCHP = 40  # padded chunk-dim stride (80B, 8B-aligned) for k-major tiles

_CACHE = {}


def _patch_act_tables():
    """Steer bacc's ACT table-set placement to the one set that covers
    every function we use (ln/exp/square/copy) so the kernel pays a single
    ACT_TABLE_LOAD instead of thrashing between per-anchor sets."""
    if _CACHE.get("act_patched"):
        return
    from concourse import bacc, mybir

    orig = bacc.get_activation_tables
    AF = mybir.ActivationFunctionType
    combo = "natural_log_exp_and_others"

    def patched(arch):
        t = {k: set(v) for k, v in orig(arch).items()}
        if combo in t:
            for name in t:
                if name != combo:
                    t[name] = t[name] - {AF.Ln, AF.Exp, AF.Copy, AF.Square}
        return t

    bacc.get_activation_tables = patched
    _CACHE["act_patched"] = True


def _build_nc():
    import concourse.tile as tile
    from concourse import bacc, mybir

    _patch_act_tables()

    nc = bacc.Bacc(
        "TRN2",
        target_bir_lowering=False,
        debug=False,
        enable_asserts=False,
        num_devices=NCORES,
    )
    x_ap = nc.dram_tensor(
        "x", [NS, D, HW], mybir.dt.bfloat16, kind="ExternalInput"
    ).ap()
    idw_ap = nc.dram_tensor(
        "idw", [D, 192], mybir.dt.bfloat16, kind="ExternalInput"
    ).ap()
    id8_ap = nc.dram_tensor(
        "id8", [NS, NS], mybir.dt.float32, kind="ExternalInput"
    ).ap()
    cent_ap = nc.dram_tensor(
        "cent", [K, D], mybir.dt.float32, kind="ExternalInput"
    ).ap()
    out_ap = nc.dram_tensor(
        "out", [NS, K, D], mybir.dt.float32, kind="ExternalOutput"
    ).ap()

    with tile.TileContext(nc) as tc:
        with ExitStack() as ctx:
            _body(ctx, tc, out_ap, x_ap, cent_ap, idw_ap, id8_ap)
    nc.compile()
    return nc


def _body(ctx, tc, out_ap, x_ap, cent_ap, idw_ap, id8_ap):
    import concourse.bass as bass
    from concourse import mybir

    nc = tc.nc
    f32 = mybir.dt.float32
    f32r = mybir.dt.float32r
    bf16 = mybir.dt.bfloat16
    AF = mybir.ActivationFunctionType
    ALU = mybir.AluOpType
    X_AX = mybir.AxisListType.X

    singles = ctx.enter_context(tc.tile_pool(name="singles", bufs=1))
    xpool = ctx.enter_context(tc.tile_pool(name="xpool", bufs=3))
    xtrpool = ctx.enter_context(tc.tile_pool(name="xtrpool", bufs=4))
    lgtpool = ctx.enter_context(tc.tile_pool(name="lgtpool", bufs=2))
    ebpool = ctx.enter_context(tc.tile_pool(name="ebpool", bufs=2))
    sbtpool = ctx.enter_context(tc.tile_pool(name="sbtpool", bufs=2))
    scrpool = ctx.enter_context(tc.tile_pool(name="scrpool", bufs=2))
    smalls = ctx.enter_context(tc.tile_pool(name="smalls", bufs=6))
    tails = ctx.enter_context(tc.tile_pool(name="tails", bufs=1))
    pp_xt = ctx.enter_context(tc.tile_pool(name="pp_xt", bufs=2, space="PSUM"))
    pp_acc = ctx.enter_context(tc.tile_pool(name="pp_acc", bufs=1, space="PSUM"))
    pp_tiny = ctx.enter_context(tc.tile_pool(name="pp_tiny", bufs=1, space="PSUM"))

    def bcast(ap, n):
        # append a step-0 free dim: [..., n] broadcast view
        return bass.AP(tensor=ap.tensor, offset=ap.offset, ap=list(ap.ap) + [[0, n]])

    def mid_bcast(ap, n):
        # [p, f] -> [p, n, f] with step-0 middle dim (keeps last dim packed
        # so DVE 2x mode stays enabled)
        return bass.AP(
            tensor=ap.tensor,
            offset=ap.offset,
            ap=[ap.ap[0], [0, n]] + list(ap.ap[1:]),
        )

    # constants
    # fused rhs for pass A: [identity | Wt] bf16 — one matmul yields
    # [X_c^T | logits]. bf16 moving tensor streams at 1 cycle/row with no
    # 256-col fp32r padding requirement; the f32r lhsT (X chunk) keeps X
    # exact through the transpose path.
    identwt = singles.tile([128, 192], bf16)
    nc.sync.dma_start(out=identwt[:], in_=idw_ap[:])
    id8 = singles.tile([NS, NS], f32)
    nc.sync.dma_start(out=id8[:], in_=id8_ap[:])
    cent_s = singles.tile([K, D], f32)
    nc.sync.dma_start(out=cent_s[:], in_=cent_ap[:])
    ones_col = singles.tile([K, 1], f32)
    nc.vector.memset(ones_col[:], 1.0)
    ones_row = singles.tile([1, K], f32)
    nc.vector.memset(ones_row[:], 1.0)

    # 6 chunks per PSUM tile, rows padded to 256 f32: a matmul out that
    # crosses a 2KB PSUM bank boundary mid-row ACCUMULATES onto stale bank
    # contents instead of resetting on start=True (verified on HW), so every
    # out row must sit inside a 512-f32 bank. 256-aligned rows never cross.
    GRP = 6
    groups = []
    c0 = 0
    while c0 < NCH:
        groups.append(list(range(c0, min(c0 + GRP, NCH))))
        c0 += GRP
    NG = len(groups)  # 7

    # pass-C interleave: chunk range emitted after group g of the round
    pc_slices = []
    base = 0
    for g in range(NG):
        take = (NCH - base + (NG - g) - 1) // (NG - g)
        pc_slices.append((base, base + take))
        base += take

    state = {}  # per-sample live tiles

    def emit_dma(n):
        # 64-col pad past HW so the last (64-wide) chunk's pass-A matmul
        # still writes all 128 PSUM partitions: without this, stale PSUM
        # (possibly NaN from a previous process) flows into the evacuated
        # lanes. The pad x-vectors are all-1 (finite everywhere downstream);
        # pass C only ever reads the valid 64 rows of that chunk.
        xs = xpool.tile([D, HW + 64], bf16, tag="xs")
        nc.gpsimd.memset(xs[:, HW : HW + 64], 1.0)
        if n == 0:
            # per-group pieces: group-0 matmuls start after ~1/7 of the load
            for grp in groups:
                p0 = CHUNKS[grp[0]][0]
                p1 = CHUNKS[grp[-1]][0] + CHUNKS[grp[-1]][1]
                nc.sync.dma_start(out=xs[:, p0:p1], in_=x_ap[n, :, p0:p1])
        else:
            nc.sync.dma_start(out=xs[:, 0 : HW // 2], in_=x_ap[n, :, 0 : HW // 2])
            nc.sync.dma_start(out=xs[:, HW // 2 : HW], in_=x_ap[n, :, HW // 2 :])
        state.setdefault(n, {})["xs"] = xs

    def alloc_sample(n):
        st = state.setdefault(n, {})
        # per chunk (bf16): cols 0:128 = X_c^T, col 128 = ||x_p|| (s),
        # cols 129:136 pad the row stride to 272B (16B-aligned): the DVE
        # 2-elem/cycle mode needs 8B-aligned row starts, so a 130-elem
        # (260B) stride would drop the square to 1x. pass C's rhs is the
        # contiguous [X^T | s] 129-col slice.
        st["xtr"] = xtrpool.tile([128, NCH, 136], bf16, tag="xtr", name="xtr")
        # logits, chunk-major [p, chunk, k]: pass C's lhsT (sbt) must be
        # contiguous, and every transpose (rearranged-view copy) measures
        # ~2.8ns/elem on HW — more than the 1x broadcast penalty it would
        # save — so the whole softmax side stays chunk-major.
        st["lgt"] = lgtpool.tile([128, NCH, K], bf16, tag="lgt", name="lgt")

    def emit_passA_group(n, g):
        st = state[n]
        xs = st["xs"]
        grp = groups[g]
        xt_p = pp_xt.tile([128, GRP, 256], f32, tag="xt")
        for j, c in enumerate(grp):
            p0, _ = CHUNKS[c]
            # always full 128-wide (xs is padded): same stream cost, and
            # every PSUM partition the evac reads is freshly written
            nc.tensor.matmul(
                xt_p[:, j, 0:192],
                lhsT=xs[:, p0 : p0 + 128],
                rhs=identwt[:],
                start=True,
                stop=True,
            )
        return xt_p, grp[0], len(grp)

    def emit_evacs(n, g, xt_p, gc, gn):
        st = state[n]
        # X^T -> xtr cols 0:128 (ACT); logits -> lgt k-major. GPSIMD cannot
        # read PSUM, so the logits copies split across ACT and DVE to
        # balance those two queues.
        nc.scalar.copy(st["xtr"][:, gc : gc + gn, 0:128], xt_p[:, 0:gn, 0:128])
        lsrc = xt_p[:, 0:gn, 128:192]
        if g < 4:
            nc.scalar.copy(st["lgt"][:, gc : gc + gn, :], lsrc)
        else:
            nc.vector.tensor_copy(st["lgt"][:, gc : gc + gn, :], lsrc)

    # ---- stage B: ss = rowsum(X_c^T^2), inv_s, s-col (sample lag 1) ----
    def st_sq(n):
        st = state[n]
        x2t = scrpool.tile([128, NCH, 128], bf16, tag="x2t", bufs=1)
        st["x2t"] = x2t
        xv = st["xtr"][:, :, 0:128]
        nc.vector.tensor_tensor(out=x2t[:], in0=xv, in1=xv, op=ALU.mult)

    def st_tree123(n):
        st = state[n]
        x2t = st["x2t"]
        t1 = scrpool.tile([128, NCH, 64], bf16, tag="t1", bufs=2)
        t2 = scrpool.tile([128, NCH, 32], bf16, tag="t2", bufs=2)
        t3 = scrpool.tile([128, NCH, 16], bf16, tag="t3", bufs=2)
        st["t3"] = t3
        nc.vector.tensor_tensor(
            out=t1[:], in0=x2t[:, :, 0:64], in1=x2t[:, :, 64:128], op=ALU.add
        )
        nc.vector.tensor_tensor(
            out=t2[:], in0=t1[:, :, 0:32], in1=t1[:, :, 32:64], op=ALU.add
        )
        nc.vector.tensor_tensor(
            out=t3[:], in0=t2[:, :, 0:16], in1=t2[:, :, 16:32], op=ALU.add
        )

    def st_tree4567(n):
        st = state[n]
        t3 = st["t3"]
        t4 = scrpool.tile([128, NCH, 8], bf16, tag="t4", bufs=2)
        t5 = scrpool.tile([128, NCH, 4], bf16, tag="t5", bufs=2)
        t6 = scrpool.tile([128, NCH, 2], bf16, tag="t6", bufs=2)
        ss = smalls.tile([128, NCH], f32, tag="ss")
        st["ss"] = ss
        nc.vector.tensor_tensor(
            out=t4[:], in0=t3[:, :, 0:8], in1=t3[:, :, 8:16], op=ALU.add
        )
        nc.vector.tensor_tensor(
            out=t5[:], in0=t4[:, :, 0:4], in1=t4[:, :, 4:8], op=ALU.add
        )
        nc.vector.tensor_tensor(
            out=t6[:], in0=t5[:, :, 0:2], in1=t5[:, :, 2:4], op=ALU.add
        )
        nc.vector.tensor_tensor(
            out=ss[:], in0=t6[:, :, 0], in1=t6[:, :, 1], op=ALU.add
        )

    def st_ischain(n):
        st = state[n]
        lns = smalls.tile([128, NCH], f32, tag="lns")
        isb = smalls.tile([128, NCH], bf16, tag="isb")
        st["isb"] = isb
        nc.scalar.activation(lns[:], st["ss"][:], AF.Ln)
        nc.scalar.activation(isb[:], lns[:], AF.Exp, scale=-0.5)

    def st_scol(n):
        st = state[n]
        # s = ss * inv_s = ||x_p||, into col 128 of each xtr chunk
        nc.gpsimd.tensor_tensor(
            out=st["xtr"][:, :, 128], in0=st["ss"][:], in1=st["isb"][:], op=ALU.mult
        )

    # ---- stage C: scaled logits, chunk-major (Pool: its only sizable
    # SBUF-only job; DVE/ACT are saturated by evac+trees) ----
    def st_slg(n):
        st = state[n]
        st["slgt"] = scrpool.tile([128, NCH, K], bf16, tag="slgt", name="slgt", bufs=2)
        nc.gpsimd.tensor_tensor(
            out=st["slgt"][:],
            in0=st["lgt"][:],
            in1=bcast(st["isb"][:], K),
            op=ALU.mult,
        )

    # ---- stage D: exp (one whole-sample ACT instruction) ----
    def st_exp(n):
        st = state[n]
        st["et"] = ebpool.tile([128, NCH, K], bf16, tag="et", name="et")
        nc.scalar.activation(st["et"][:], st["slgt"][:], AF.Exp)

    # ---- stage E: Z (k-halving tree), 1/Z, t = inv_s/Z, sb (lag 2) ----
    def st_zchain(n):
        st = state[n]
        et = st["et"]
        z1 = scrpool.tile([128, NCH, 32], bf16, tag="z1", bufs=2)
        z2 = scrpool.tile([128, NCH, 16], bf16, tag="z2", bufs=2)
        z3 = scrpool.tile([128, NCH, 8], bf16, tag="z3", bufs=2)
        z4 = scrpool.tile([128, NCH, 4], bf16, tag="z4", bufs=2)
        z5 = scrpool.tile([128, NCH, 2], bf16, tag="z5", bufs=2)
        zz = smalls.tile([128, NCH], f32, tag="zz")
        st["zz"] = zz
        nc.vector.tensor_tensor(
            out=z1[:], in0=et[:, :, 0:32], in1=et[:, :, 32:64], op=ALU.add
        )
        nc.vector.tensor_tensor(
            out=z2[:], in0=z1[:, :, 0:16], in1=z1[:, :, 16:32], op=ALU.add
        )
        nc.vector.tensor_tensor(
            out=z3[:], in0=z2[:, :, 0:8], in1=z2[:, :, 8:16], op=ALU.add
        )
        nc.vector.tensor_tensor(
            out=z4[:], in0=z3[:, :, 0:4], in1=z3[:, :, 4:8], op=ALU.add
        )
        nc.vector.tensor_tensor(
            out=z5[:], in0=z4[:, :, 0:2], in1=z4[:, :, 2:4], op=ALU.add
        )
        nc.vector.tensor_tensor(
            out=zz[:], in0=z5[:, :, 0], in1=z5[:, :, 1], op=ALU.add
        )

    def st_recip(n):
        st = state[n]
        rr = smalls.tile([128, NCH], f32, tag="rr")
        st["rr"] = rr
        nc.vector.reciprocal(rr[:], st["zz"][:])

    def st_tsc(n):
        st = state[n]
        tsc = smalls.tile([128, NCH], bf16, tag="tsc")
        st["tsc"] = tsc
        nc.vector.tensor_tensor(
            out=tsc[:], in0=st["isb"][:], in1=st["rr"][:], op=ALU.mult
        )

    def st_sbt(n, half, eng):
        st = state[n]
        if "sbt" not in st:
            st["sbt"] = sbtpool.tile([128, NCH, K], bf16, tag="sbt", name="sbt")
        c0, c1 = (0, NCH // 2) if half == 0 else (NCH // 2, NCH)
        eng.tensor_tensor(
            out=st["sbt"][:, c0:c1, :],
            in0=st["et"][:, c0:c1, :],
            in1=bcast(st["tsc"][:, c0:c1], K),
            op=ALU.mult,
        )

    cstate = {}  # open accumulation tiles for interleaved pass C

    def emit_passC_chunks(n, c0, c1):
        st = state[n]
        xtr, sbt = st["xtr"], st["sbt"]
        if n not in cstate:
            cstate[n] = pp_acc.tile([K, 129], f32, tag="acc", name="acc")
        acc_p = cstate[n]
        for c in range(c0, min(c1, NCH)):
            p0, w = CHUNKS[c]
            nc.tensor.matmul(
                acc_p[:, :],
                lhsT=sbt[:w, c, :],
                rhs=xtr[:w, c, 0:129],
                start=(c == 0),
                stop=(c == NCH - 1),
            )

    def finish_passC(n):
        acc_p = cstate.pop(n)
        state.pop(n)
        # evacuate [sum_sa | agg] in one copy
        nc.scalar.copy(comb_all[:, n, :], acc_p[:, 0 : D + 1])

    # batched across all samples; acc layout is [agg | sum_sa]
    comb_all = tails.tile([K, NS, D + 1], f32)
    agg_all = comb_all[:, :, 0:D]
    ssa_all = comb_all[:, :, D]

    def emit_tail(n0, n1):
        nn = n1 - n0
        agg_h = agg_all[:, n0:n1, :]
        ssa_h = ssa_all[:, n0:n1]
        vl = tails.tile([K, nn, D], f32, tag="t_vl", bufs=2)
        vsq = tails.tile([K, nn * D], f32, tag="t_vsq", bufs=2)
        q = tails.tile([K, nn], f32, tag="t_q", bufs=2)
        qm = tails.tile([K, nn], f32, tag="t_qm", bufs=2)
        isq = tails.tile([K, nn], f32, tag="t_isq", bufs=2)
        isq2 = tails.tile([K, nn], f32, tag="t_isq2", bufs=2)
        u = tails.tile([K, nn], f32, tag="t_u", bufs=2)
        gisr = tails.tile([1, nn], f32, tag="t_gisr", bufs=2)
        gb = tails.tile([K, nn], f32, tag="t_gb", bufs=2)
        sall = tails.tile([K, nn], f32, tag="t_s", bufs=2)
        vf = tails.tile([K, nn, D], f32, tag="t_vf", bufs=2)

        # vl = agg - ssa * cent
        nc.gpsimd.tensor_tensor(
            out=vl[:], in0=bcast(ssa_h, D), in1=mid_bcast(cent_s[:], nn), op=ALU.mult
        )
        nc.vector.tensor_tensor(out=vl[:], in0=agg_h, in1=vl[:], op=ALU.subtract)
        # q = rowsum(vl^2) per (k, n)
        vsqv = vsq[:].rearrange("k (n d) -> k n d", n=nn)
        nc.scalar.activation(vsqv, vl[:], AF.Square)
        nc.vector.tensor_reduce(out=q[:], in_=vsqv, axis=X_AX, op=ALU.add)
        nc.vector.tensor_scalar_max(qm[:], q[:], 1e-24)
        lq = tails.tile([K, nn], f32, tag="t_lq", bufs=2)
        nc.scalar.activation(lq[:], qm[:], AF.Ln)
        nc.scalar.activation(isq[:], lq[:], AF.Exp, scale=-0.5)
        # g = sum_k q_k * isq_k^2  (per sample)
        nc.vector.tensor_tensor(out=isq2[:], in0=isq[:], in1=isq[:], op=ALU.mult)
        nc.vector.tensor_tensor(out=u[:], in0=q[:], in1=isq2[:], op=ALU.mult)
        g_p = pp_tiny.tile([NS, 1], f32, tag="tiny")
        nc.tensor.matmul(
            g_p[:nn, :], lhsT=u[:], rhs=ones_col[:], start=True, stop=True
        )
        # gis = g^-0.5 -> transpose to a row -> broadcast over k partitions
        gm = tails.tile([nn, 1], f32, tag="t_gm", bufs=2)
        nc.vector.tensor_scalar_max(gm[:], g_p[:nn, :], 1e-24)
        gis = tails.tile([nn, 1], f32, tag="t_gis", bufs=2)
        lgm = tails.tile([nn, 1], f32, tag="t_lgm", bufs=2)
        nc.scalar.activation(lgm[:], gm[:], AF.Ln)
        nc.scalar.activation(gis[:], lgm[:], AF.Exp, scale=-0.5)
        gr_p = pp_tiny.tile([1, NS], f32, tag="tiny")
        nc.tensor.matmul(
            gr_p[:, :nn],
            lhsT=gis[:],
            rhs=id8[:nn, 0:nn],
            start=True,
            stop=True,
        )
        nc.vector.tensor_copy(gisr[:], gr_p[:, :nn])
        gb_p = pp_tiny.tile([K, NS], f32, tag="tiny")
        nc.tensor.matmul(
            gb_p[:, :nn], lhsT=ones_row[:], rhs=gisr[:], start=True, stop=True
        )
        nc.vector.tensor_copy(gb[:], gb_p[:, :nn])
        # s = isq * gb; vf = vl * s
        nc.vector.tensor_tensor(out=sall[:], in0=isq[:], in1=gb[:], op=ALU.mult)
        nc.gpsimd.tensor_tensor(out=vf[:], in0=vl[:], in1=bcast(sall[:], D), op=ALU.mult)
        nc.sync.dma_start(
            out=out_ap.rearrange("n k d -> k n d")[:, n0:n1, :], in_=vf[:]
        )

    # 5-stage software pipeline, one stage lag per round; every stage
    # consumes only tensors finished a round earlier:
    #   round r: DMA prefetch(r+2) | passA+evacs(r) | B: ss/inv_s/slg(r-1) |
    #   D: exp(r-2, ACT) | E: Z/recip/sb(r-2, DVE) | passC(r-3) on PE.
    emit_dma(0)
    emit_dma(1)
    pending_fin = None
    for r in range(NS + 3):
        na = r if r < NS else None
        if na is not None and r + 2 < NS:
            emit_dma(r + 2)
        if na is not None:
            alloc_sample(na)
        nb = r - 1 if 0 <= r - 1 < NS else None
        sd = r - 2 if 0 <= r - 2 < NS else None
        pe = r - 3 if 0 <= r - 3 < NS else None
        # finish the previous round's accumulation at the HEAD of this
        # round's queues so the acc bank frees before passC(pe) needs it
        if pending_fin is not None:
            finish_passC(pending_fin)
            if pending_fin % 2 == 1:
                emit_tail(pending_fin - 1, pending_fin + 1)
            pending_fin = None
        for g in range(NG):
            # passC first: it is always ready (sbt a round old), so the PE
            # drains it while a late xs DMA finishes instead of idling
            if pe is not None:
                emit_passC_chunks(pe, *pc_slices[g])
            if na is not None:
                xt_p, gc, gn = emit_passA_group(na, g)
                emit_evacs(na, g, xt_p, gc, gn)
            if g == 0 and nb is not None:
                st_sq(nb)
            elif g == 1 and nb is not None:
                st_tree123(nb)
            elif g == 2 and nb is not None:
                st_tree4567(nb)
            elif g == 3:
                if nb is not None:
                    st_ischain(nb)
                    st_scol(nb)
            elif g == 5:
                if sd is not None:
                    st_exp(sd)
                if nb is not None:
                    st_slg(nb)
        if sd is not None:
            # round tail: exp(sd) just finished on ACT; chain Z -> 1/Z -> t
            # -> sb here so sb is ready when passC(sd) starts next round
            st_zchain(sd)
            st_recip(sd)
            st_tsc(sd)
            st_sbt(sd, 0, nc.vector)
            st_sbt(sd, 1, nc.vector)
        if pe is not None:
            pending_fin = pe
    if pending_fin is not None:
        n_last = pending_fin
        finish_passC(n_last)
        emit_tail(n_last - 1, n_last + 1)


def kernel(x, conv_w, centroids):
    from concourse.bass_utils import run_bass_kernel_spmd
    import ml_dtypes

    if "nc" not in _CACHE:
        _CACHE["nc"] = _build_nc()
    nc = _CACHE["nc"]

    x = np.ascontiguousarray(
        np.asarray(x, dtype=np.float32).reshape(N, D, HW).astype(ml_dtypes.bfloat16)
    )
    wt = np.ascontiguousarray(np.asarray(conv_w, dtype=np.float32).T)
    cent = np.ascontiguousarray(np.asarray(centroids, dtype=np.float32))
    idw = np.zeros((D, 192), dtype=np.float32)
    idw[:, 0:128] = np.eye(D, dtype=np.float32)
    idw[:, 128:192] = wt
    idw = idw.astype(ml_dtypes.bfloat16)
    id8 = np.eye(NS, dtype=np.float32)
    in_maps = [
        {"x": x[i * NS : (i + 1) * NS], "cent": cent, "idw": idw, "id8": id8}
        for i in range(NCORES)
    ]
    res = run_bass_kernel_spmd(nc, in_maps, core_ids=list(range(NCORES))).results
    out = np.concatenate([r["out"].reshape(NS, K * D) for r in res], axis=0)
    return out


if __name__ == "__main__":
    rng = np.random.default_rng(0)
    xs = rng.standard_normal((N, D, 60, 80), dtype=np.float32)
    cw = (rng.standard_normal((K, D)) * 0.1).astype(np.float32)
    ct = rng.random((K, D), dtype=np.float32)
    o = kernel(x=xs, conv_w=cw, centroids=ct)
    print("kernel out", o.shape, o.dtype, np.abs(o).max())
